# revision 13
# baseline (speedup 1.0000x reference)
"""Causal self-attention (BS=4, SL=2048, NE=1024, NH=16) on 8 trn2 NeuronCores.

Sharding (uniform SPMD program on all 8 cores):
  core c -> batch b = c//2, head-group g = c%2 (8 of 16 heads, 512 feats).
  Each core: QKV proj for its heads (full 2048 rows of its batch),
  causal attention for its 8 heads, then pairwise AllGather of y
  (cores 2b/2b+1), and out-proj for a 512-column half of the output.
  Host reassembles [4, 2048, 1024] from the 8 [2048, 512] halves.

v2 optimizations over the first working version:
  - x is pre-transposed on the HOST, so every x^T tile is a plain row
    DMA (the DMA-transpose ring was 50%-busy serialized before).
  - K bias dropped (softmax is invariant to a per-query additive shift)
    and V bias folded into the out-proj bias on the host (y = sum w_k
    (v_k + bv) = sum w_k v_k + bv since sum w_k = 1).
  - softmax 1/denom via reciprocal_approx_fast (5x faster DVE op;
    the exact [1,512] reciprocal was 3.35us x32 = 107us of DVE busy).
  - out-proj bias via a precomputed broadcast tile + tensor_add epilogue
    instead of a [1,128]x[128,512] PE matmul per block.
  - software-pipelined attention inner loop (S-matmul pairs run two
    ahead of PV pairs, so PV never head-of-line-blocks the PE queue
    behind the Exp activation), with projection/out-proj micro-ops
    (2 matmuls each) interleaved between slots to fill PE gaps.
  - out-projection of chunk i is interleaved into attention of panel
    i+2 instead of running serialized at the end.
  - fp16 output (cast back to fp32 on host).

Matmul operands in fp16 (full PE rate, fp32 PSUM accumulate).
Attention computed in S^T = K @ Q^T layout so that:
  - PV needs no transposes: Y^T[65,q] += [V|1]^T @ expS^T (row 64 = denom)
  - softmax normalization via gpsimd partition_broadcast of 1/denom
"""

import sys

if "/opt/trn_rl_repo" not in sys.path:
    sys.path.insert(0, "/opt/trn_rl_repo")

import numpy as np

import concourse.bass as bass
import concourse.mybir as mybir
import concourse.tile as tile
from concourse import bacc
from concourse.bass_utils import run_bass_kernel_spmd

F32 = mybir.dt.float32
F16 = mybir.dt.float16

# problem dims (hardcoded per spec)
BS, SL, NE, NH = 4, 2048, 1024, 16
HD = 64
N_CORES = 8


def build_nc(sl=SL, ne=NE, nh=NH, fake_collective=False):
    """Build the per-core Bass program. All 8 cores run this identically."""
    H = nh // 2          # local heads per core
    F = H * HD           # local feats (q/k/v width per core)
    FG = F // 128        # feat groups of 128 (2 heads each)
    CH = ne // 128       # contraction chunks for the projections
    PANEL = 512          # q-panel width
    NP = sl // PANEL     # number of q panels
    NKB = sl // 128      # number of 128-row k blocks
    OUTW = ne // 2       # out-proj columns computed per core
    VW = H * 65          # V' width (65-stride per head: 64 V cols + ones)
    N_CC = NP            # collective chunk == q panel
    SPAN = PANEL

    nc = bacc.Bacc("TRN2", target_bir_lowering=False, num_devices=N_CORES)

    xt = nc.dram_tensor("xt", [ne, sl], F16, kind="ExternalInput")
    wq = nc.dram_tensor("wq", [ne, F], F16, kind="ExternalInput")
    wk = nc.dram_tensor("wk", [ne, F], F16, kind="ExternalInput")
    wv = nc.dram_tensor("wv", [ne, F], F16, kind="ExternalInput")
    bq = nc.dram_tensor("bq", [F], F32, kind="ExternalInput")
    wo = nc.dram_tensor("wo", [ne, OUTW], F16, kind="ExternalInput")
    bo = nc.dram_tensor("bo", [OUTW], F32, kind="ExternalInput")
    out = nc.dram_tensor("out", [sl, OUTW], F16, kind="ExternalOutput")

    tri_dram = nc.inline_tensor(
        np.triu(np.ones((128, 128), dtype=np.float16)), name="tri_c")

    with tile.TileContext(nc) as tc:
        with (
            tc.tile_pool(name="consts", bufs=1) as consts,
            tc.tile_pool(name="xt", bufs=3) as xtp,
            tc.tile_pool(name="qt", bufs=2) as qtp,
            tc.tile_pool(name="persist", bufs=1) as persist,
            tc.tile_pool(name="es", bufs=8) as esp,
            tc.tile_pool(name="ny", bufs=3) as nyp,
            tc.tile_pool(name="misc", bufs=2) as misc,
            tc.tile_pool(name="psum", bufs=1, space="PSUM") as psp,
            tc.tile_pool(name="dram", bufs=1, space="DRAM") as dram,
        ):
            # ---- panel-0 x^T tiles + first weights so proj starts ASAP ----
            xT0 = [xtp.tile([128, PANEL], F16, tag=f"xt{c}", name=f"xT0_{c}")
                   for c in range(CH)]
            WQ = [persist.tile([128, F], F16, tag=f"wq{c}", name=f"WQ{c}")
                  for c in range(CH)]
            WK = [persist.tile([128, F], F16, tag=f"wk{c}", name=f"WK{c}")
                  for c in range(CH)]
            WV = [persist.tile([128, F], F16, tag=f"wv{c}", name=f"WV{c}")
                  for c in range(CH)]
            WO = [persist.tile([128, OUTW], F16, tag=f"wo{c}", name=f"WO{c}")
                  for c in range(CH)]
            for c in (0, 1):
                nc.sync.dma_start(out=xT0[c],
                                  in_=xt[c * 128:(c + 1) * 128, 0:PANEL])
                nc.sync.dma_start(out=WQ[c], in_=wq[c * 128:(c + 1) * 128, :])
            bqt = consts.tile([128, FG], F32)
            nc.sync.dma_start(out=bqt, in_=bq.rearrange("(g p) -> p g", p=128))
            for c in range(2, CH):
                nc.sync.dma_start(out=xT0[c],
                                  in_=xt[c * 128:(c + 1) * 128, 0:PANEL])
                nc.sync.dma_start(out=WQ[c], in_=wq[c * 128:(c + 1) * 128, :])
            for c in range(CH):
                sl_c = slice(c * 128, (c + 1) * 128)
                nc.sync.dma_start(out=WK[c], in_=wk[sl_c, :])
                nc.sync.dma_start(out=WV[c], in_=wv[sl_c, :])

            # ---- constants ----
            tri = consts.tile([128, 128], F16)
            nc.sync.dma_start(out=tri, in_=tri_dram[:])
            bo_row = consts.tile([1, OUTW], F32)
            nc.sync.dma_start(out=bo_row,
                              in_=bo.rearrange("(a n) -> a n", a=1))
            bo_bc = consts.tile([128, OUTW], F32)
            nc.gpsimd.partition_broadcast(bo_bc, bo_row)

            for c in range(CH):
                nc.sync.dma_start(out=WO[c], in_=wo[c * 128:(c + 1) * 128, :])

            # ---- persistent attention operands ----
            KT = [persist.tile([128, sl], F16, tag=f"kt{f}", name=f"KT{f}")
                  for f in range(FG)]
            VP = [persist.tile([128, VW], F16, tag=f"vp{k}", name=f"VP{k}")
                  for k in range(NKB)]

            y_local = dram.tile([N_CC, F, SPAN], F16)
            # AllGather split into head-halves: half A (heads 0-3) fires
            # mid-panel so only half B's 256KB transfer sits on the tail
            y_allA = dram.tile([N_CC, 2, F // 2, SPAN], F16)
            y_allB = dram.tile([N_CC, 2, F // 2, SPAN], F16)

            QTs = {}
            y_rows = {}

            def emit_xT(p):
                tiles = [xtp.tile([128, PANEL], F16, tag=f"xt{c}",
                                  name=f"xT{c}") for c in range(CH)]
                for c in range(CH):
                    nc.sync.dma_start(
                        out=tiles[c],
                        in_=xt[c * 128:(c + 1) * 128,
                               p * PANEL:(p + 1) * PANEL])
                return tiles

            # ---------- projection micro-ops (2 matmuls each) ----------
            def proj_micros(p, xT):
                QT = [qtp.tile([128, PANEL], F16, tag=f"qt{f}",
                               name=f"QT{f}") for f in range(FG)]
                QTs[p] = QT
                micros = []

                def qk_mms(f, wtiles, st, c0):
                    def go():
                        if 'ps' not in st:
                            st['ps'] = psp.tile([128, PANEL], F32, tag="acc",
                                                bufs=2, name="ps_a")
                        for c in (c0, c0 + 1):
                            nc.tensor.matmul(
                                st['ps'], wtiles[c][:, f * 128:(f + 1) * 128],
                                xT[c], start=(c == 0), stop=(c == CH - 1))
                    return go

                def q_epi(f, st):
                    def go():
                        nc.vector.tensor_scalar_add(
                            QT[f], st['ps'], bqt[:, f:f + 1])
                    return go

                def k_epi(f, st):
                    def go():
                        nc.vector.tensor_copy(
                            KT[f][:, p * PANEL:(p + 1) * PANEL], st['ps'])
                    return go

                def v_mms(sub, st, c0):
                    def go():
                        if 'ps' not in st:
                            st['ps'] = psp.tile([128, F], F32, tag="acc",
                                                bufs=2, name="ps_v")
                        for c in (c0, c0 + 1):
                            nc.tensor.matmul(
                                st['ps'], xT[c][:, sub * 128:(sub + 1) * 128],
                                WV[c], start=(c == 0), stop=(c == CH - 1))
                    return go

                def v_epi(sub, st):
                    def go():
                        kb = p * 4 + sub
                        vp3 = VP[kb].rearrange("p (h e) -> p h e", e=65)
                        nc.vector.memset(vp3[:, :, 64:65], 1.0)
                        nc.vector.tensor_copy(
                            vp3[:, :, 0:64],
                            st['ps'].rearrange("p (h d) -> p h d", d=64))
                    return go

                for f in range(FG):
                    stq, stk = {}, {}
                    for c0 in range(0, CH, 2):
                        micros.append(qk_mms(f, WQ, stq, c0))
                    micros.append(q_epi(f, stq))
                    for c0 in range(0, CH, 2):
                        micros.append(qk_mms(f, WK, stk, c0))
                    micros.append(k_epi(f, stk))
                for sub in range(4):
                    stv = {}
                    for c0 in range(0, CH, 2):
                        micros.append(v_mms(sub, stv, c0))
                    micros.append(v_epi(sub, stv))
                return micros

            # ---------- attention slots (software-pipelined) ----------
            def head_slots(p, h):
                f, row = h // 2, (h % 2) * 64
                QT = QTs[p]
                nkb_p = 4 * p + 4
                npairs = nkb_p // 2
                st = {}

                def spair(j):
                    def go():
                        ps_s = psp.tile([128, 2 * PANEL], F32, tag="s",
                                        bufs=2, name="ps_s")
                        es = esp.tile([128, 2 * PANEL], F16, tag="es",
                                      name="es")
                        offs = []
                        for jj in (0, 1):
                            kb = 2 * j + jj
                            d = max(0, (kb - 4 * p) * 128)
                            offs.append((kb, d, PANEL - d, jj * PANEL))
                            nc.tensor.matmul(
                                ps_s[:, jj * PANEL:jj * PANEL + PANEL - d],
                                KT[f][row:row + 64,
                                      kb * 128:(kb + 1) * 128],
                                QT[f][row:row + 64, d:PANEL])
                        st[('es', j)] = es
                        st[('offs', j)] = offs
                        if 2 * j >= 4 * p:
                            for kb, d, n, o in offs:
                                nc.scalar.activation(
                                    es[:, o:o + n], ps_s[:, o:o + n],
                                    mybir.ActivationFunctionType.Exp)
                                nc.vector.tensor_mul(
                                    es[:, o:o + 128], es[:, o:o + 128], tri)
                        else:
                            nc.scalar.activation(
                                es, ps_s, mybir.ActivationFunctionType.Exp)
                    return go

                def pvpair(j):
                    def go():
                        if 'ps_y' not in st:
                            st['ps_y'] = psp.tile([65, PANEL], F32, tag="y",
                                                  bufs=2, name="ps_y")
                        ps_y = st['ps_y']
                        es = st.pop(('es', j))
                        for kb, d, n, o in st.pop(('offs', j)):
                            nc.tensor.matmul(
                                ps_y[:, d:PANEL],
                                VP[kb][:, h * 65:h * 65 + 65],
                                es[:, o:o + n],
                                start=(kb == 0), stop=(kb == nkb_p - 1))
                    return go

                def norm():
                    ps_y = st['ps_y']
                    den_s = misc.tile([1, PANEL], F32, tag="den",
                                      name="den")
                    nc.scalar.copy(den_s, ps_y[64:65, :])
                    recip = misc.tile([1, PANEL], F32, tag="recip",
                                      name="recip")
                    nc.vector.reciprocal_approx_fast(out=recip, in_=den_s)
                    bc = misc.tile([64, PANEL], F32, tag="bc", bufs=3,
                                   name="bc")
                    nc.gpsimd.partition_broadcast(bc, recip)
                    nY = nyp.tile([64, PANEL], F16, tag=f"ny{h}", name="nY")
                    nc.vector.tensor_mul(nY, ps_y[0:64, :], bc)
                    nc.sync.dma_start(
                        out=y_local[p, h * 64:(h + 1) * 64, :], in_=nY)

                slots = []
                for j in range(npairs):
                    slots.append(spair(j))
                    if j >= 2:
                        slots.append(pvpair(j - 2))
                slots.append(pvpair(npairs - 2))
                slots.append(pvpair(npairs - 1))
                return slots, norm

            def half_slots(p, heads):
                # delay each head's norm until 2 slots into the NEXT head:
                # the norm's den-copy otherwise sits in the ACT FIFO ahead
                # of the next head's first Exp and stalls its PV pair
                out = []
                pending = None
                for h in heads:
                    body, norm = head_slots(p, h)
                    out.extend(body[:2])
                    if pending is not None:
                        out.append(pending)
                    out.extend(body[2:])
                    pending = norm
                out.append(pending)
                return out

            # ---------- collective + out-proj ----------
            def emit_cc_half(i, half):
                fh = F // 2
                y_in = y_local[i][half * fh:(half + 1) * fh, :]
                y_out = (y_allA if half == 0 else y_allB)[i]
                if fake_collective:
                    nc.sync.dma_start(out=y_out[0], in_=y_in)
                    nc.sync.dma_start(out=y_out[1], in_=y_in)
                else:
                    nc.gpsimd.collective_compute(
                        "AllGather",
                        mybir.AluOpType.bypass,
                        replica_groups=[[0, 1], [2, 3], [4, 5], [6, 7]],
                        ins=[y_in.opt()],
                        outs=[y_out.opt()],
                    )
                # one DMA for all 4 c-chunks of this half:
                # t[p, (g a), s] <- y_out[g, a*128+p, s]
                t = misc.tile([128, 4, PANEL], F16, tag=f"y_rows{half}",
                              bufs=2, name=f"y_rows{half}")
                nc.sync.dma_start(
                    out=t,
                    in_=y_out.rearrange("g (a p) s -> p (g a) s", p=128))
                # half 0 carries c = 0,1 (mine) and 4,5 (partner); half 1
                # carries c = 2,3 and 6,7
                cm = y_rows.setdefault(i, {})
                for j, c in enumerate((0, 1, 4, 5) if half == 0
                                      else (2, 3, 6, 7)):
                    cm[c] = (t, j)

            def outproj_micros(i):
                y_row = y_rows[i]
                micros = []

                def mms(sb4, st, c0):
                    def go():
                        if 'ps' not in st:
                            st['ps'] = psp.tile([128, OUTW], F32, tag="acc",
                                                bufs=2, name="ps_o")
                        for c in (c0, c0 + 1):
                            t, j = y_row[c]
                            nc.tensor.matmul(
                                st['ps'],
                                t[:, j, sb4 * 128:(sb4 + 1) * 128],
                                WO[c], start=(c == 0), stop=(c == CH - 1))
                    return go

                def epi(sb4, st):
                    def go():
                        sb = i * (PANEL // 128) + sb4
                        o_t = misc.tile([128, OUTW], F16, tag="o_t", bufs=3,
                                        name="o_t")
                        nc.vector.tensor_add(o_t, st['ps'], bo_bc)
                        # split across DMA queues; 4-way for the final
                        # chunk whose writes are the span's drain
                        r = slice(sb * 128, (sb + 1) * 128)
                        ways = 4 if i == N_CC - 1 else 2
                        w = OUTW // ways
                        for k in range(ways):
                            nc.sync.dma_start(
                                out=out[r, k * w:(k + 1) * w],
                                in_=o_t[:, k * w:(k + 1) * w])
                    return go

                for sb4 in range(PANEL // 128):
                    st = {}
                    for c0 in range(0, CH, 2):
                        micros.append(mms(sb4, st, c0))
                    micros.append(epi(sb4, st))
                return micros

            def interleave(slots, fillers):
                n, m = len(slots), len(fillers)
                fi = 0
                for si, s in enumerate(slots):
                    s()
                    while fi < m and fi * n <= (si + 1) * m:
                        fillers[fi]()
                        fi += 1
                while fi < m:
                    fillers[fi]()
                    fi += 1

            # ---------- schedule ----------
            def run_round(pa, fillers):
                slotsA = half_slots(pa, range(0, H // 2))
                slotsB = half_slots(pa, range(H // 2, H))
                kA = len(fillers) * len(slotsA) // (len(slotsA) +
                                                    len(slotsB))
                interleave(slotsA, fillers[:kA])
                emit_cc_half(pa, 0)
                interleave(slotsB, fillers[kA:])
                emit_cc_half(pa, 1)
                QTs.pop(pa, None)

            # round 0: proj(0) straight
            for mo in proj_micros(0, xT0):
                mo()
            # rounds 1..NP-1: attn(p-1) x [proj(p) + outproj(p-2)]
            for p in range(1, NP):
                fillers = proj_micros(p, emit_xT(p))
                if p >= 2:
                    fillers += outproj_micros(p - 2)
                run_round(p - 1, fillers)
            # final round: attn(NP-1) x outproj(NP-2)
            run_round(NP - 1, outproj_micros(NP - 2))
            for mo in outproj_micros(NP - 1):
                mo()

    nc.compile()
    return nc


def shard_inputs(x, mask, Wqkv, bqkv, Wo, bo, sl=SL, ne=NE, nh=NH):
    """Host-side sharding: returns in_maps for the 8 cores."""
    H = nh // 2
    F = H * HD
    scale = 1.0 / np.sqrt(HD)
    bv_full = bqkv[2 * ne:3 * ne]
    bo_eff = bo + bv_full @ Wo  # V bias folded through the out projection
    in_maps = []
    for c in range(N_CORES):
        b, g = c // 2, c % 2
        qc = slice(g * F, (g + 1) * F)
        kc = slice(ne + g * F, ne + (g + 1) * F)
        vc = slice(2 * ne + g * F, 2 * ne + (g + 1) * F)
        oc = slice(g * (ne // 2), (g + 1) * (ne // 2))
        in_maps.append({
            "xt": np.ascontiguousarray(x[b].T).astype(np.float16),
            "wq": (np.ascontiguousarray(Wqkv[:, qc]) * scale).astype(np.float16),
            "wk": np.ascontiguousarray(Wqkv[:, kc]).astype(np.float16),
            "wv": np.ascontiguousarray(Wqkv[:, vc]).astype(np.float16),
            "bq": (np.ascontiguousarray(bqkv[qc]) * scale).astype(np.float32),
            "wo": np.ascontiguousarray(Wo[:, oc]).astype(np.float16),
            "bo": np.ascontiguousarray(bo_eff[oc]).astype(np.float32),
        })
    return in_maps


def unshard_output(results, sl=SL, ne=NE):
    out = np.empty((BS, sl, ne), dtype=np.float32)
    half = ne // 2
    for c in range(N_CORES):
        b, g = c // 2, c % 2
        out[b, :, g * half:(g + 1) * half] = results[c]["out"].astype(
            np.float32)
    return out


_NC_CACHE = {}


def kernel(x, mask, Wqkv, bqkv, Wo, bo):
    x = np.asarray(x, dtype=np.float32)
    Wqkv = np.asarray(Wqkv, dtype=np.float32)
    bqkv = np.asarray(bqkv, dtype=np.float32)
    Wo = np.asarray(Wo, dtype=np.float32)
    bo = np.asarray(bo, dtype=np.float32)
    if "nc" not in _NC_CACHE:
        _NC_CACHE["nc"] = build_nc()
    nc = _NC_CACHE["nc"]
    in_maps = shard_inputs(x, mask, Wqkv, bqkv, Wo, bo)
    res = run_bass_kernel_spmd(nc, in_maps, list(range(N_CORES)))
    return unshard_output(res.results)


# revision 16
# speedup vs baseline: 1.0537x; 1.0537x over previous
"""Causal self-attention (BS=4, SL=2048, NE=1024, NH=16) on 8 trn2 NeuronCores.

Sharding (uniform SPMD program on all 8 cores):
  core c -> batch b = c//2, head-group g = c%2 (8 of 16 heads, 512 feats).
  Each core: QKV proj for its heads (full 2048 rows of its batch),
  causal attention for its 8 heads, then pairwise AllGather of y
  (cores 2b/2b+1), and out-proj for a 512-column half of the output.
  Host reassembles [4, 2048, 1024] from the 8 [2048, 512] halves.

v2 optimizations over the first working version:
  - x is pre-transposed on the HOST, so every x^T tile is a plain row
    DMA (the DMA-transpose ring was 50%-busy serialized before).
  - K bias dropped (softmax is invariant to a per-query additive shift)
    and V bias folded into the out-proj bias on the host (y = sum w_k
    (v_k + bv) = sum w_k v_k + bv since sum w_k = 1).
  - softmax 1/denom via reciprocal_approx_fast (5x faster DVE op;
    the exact [1,512] reciprocal was 3.35us x32 = 107us of DVE busy).
  - out-proj bias via a precomputed broadcast tile + tensor_add epilogue
    instead of a [1,128]x[128,512] PE matmul per block.
  - software-pipelined attention inner loop (S-matmul pairs run two
    ahead of PV pairs, so PV never head-of-line-blocks the PE queue
    behind the Exp activation), with projection/out-proj micro-ops
    (2 matmuls each) interleaved between slots to fill PE gaps.
  - out-projection of chunk i is interleaved into attention of panel
    i+2 instead of running serialized at the end.
  - fp16 output (cast back to fp32 on host).

Matmul operands in fp16 (full PE rate, fp32 PSUM accumulate).
Attention computed in S^T = K @ Q^T layout so that:
  - PV needs no transposes: Y^T[65,q] += [V|1]^T @ expS^T (row 64 = denom)
  - softmax normalization via gpsimd partition_broadcast of 1/denom
"""

import sys

if "/opt/trn_rl_repo" not in sys.path:
    sys.path.insert(0, "/opt/trn_rl_repo")

import numpy as np

import concourse.bass as bass
import concourse.mybir as mybir
import concourse.tile as tile
from concourse import bacc
from concourse.bass_utils import run_bass_kernel_spmd

F32 = mybir.dt.float32
F16 = mybir.dt.float16

# problem dims (hardcoded per spec)
BS, SL, NE, NH = 4, 2048, 1024, 16
HD = 64
N_CORES = 8


def build_nc(sl=SL, ne=NE, nh=NH, fake_collective=False):
    """Build the per-core Bass program. All 8 cores run this identically."""
    H = nh // 2          # local heads per core
    F = H * HD           # local feats (q/k/v width per core)
    FG = F // 128        # feat groups of 128 (2 heads each)
    CH = ne // 128       # contraction chunks for the projections
    PANEL = 512          # q-panel width
    NP = sl // PANEL     # number of q panels
    NKB = sl // 128      # number of 128-row k blocks
    OUTW = ne // 2       # out-proj columns computed per core
    VW = H * 65          # V' width (65-stride per head: 64 V cols + ones)
    N_CC = NP            # collective chunk == q panel
    SPAN = PANEL

    nc = bacc.Bacc("TRN2", target_bir_lowering=False, num_devices=N_CORES)

    xt = nc.dram_tensor("xt", [ne, sl], F16, kind="ExternalInput")
    wq = nc.dram_tensor("wq", [ne, F], F16, kind="ExternalInput")
    wk = nc.dram_tensor("wk", [ne, F], F16, kind="ExternalInput")
    wv = nc.dram_tensor("wv", [ne, F], F16, kind="ExternalInput")
    bq = nc.dram_tensor("bq", [F], F32, kind="ExternalInput")
    wo = nc.dram_tensor("wo", [ne, OUTW], F16, kind="ExternalInput")
    bo = nc.dram_tensor("bo", [OUTW], F32, kind="ExternalInput")
    out = nc.dram_tensor("out", [sl, OUTW], F16, kind="ExternalOutput")

    tri_dram = nc.inline_tensor(
        np.triu(np.ones((128, 128), dtype=np.float16)), name="tri_c")

    with tile.TileContext(nc) as tc:
        with (
            tc.tile_pool(name="consts", bufs=1) as consts,
            tc.tile_pool(name="xt", bufs=3) as xtp,
            tc.tile_pool(name="qt", bufs=2) as qtp,
            tc.tile_pool(name="persist", bufs=1) as persist,
            tc.tile_pool(name="es", bufs=8) as esp,
            tc.tile_pool(name="ny", bufs=3) as nyp,
            tc.tile_pool(name="misc", bufs=2) as misc,
            tc.tile_pool(name="psum", bufs=1, space="PSUM") as psp,
            tc.tile_pool(name="dram", bufs=1, space="DRAM") as dram,
        ):
            # ---- panel-0 x^T tiles + first weights so proj starts ASAP ----
            xT0 = [xtp.tile([128, PANEL], F16, tag=f"xt{c}", name=f"xT0_{c}")
                   for c in range(CH)]
            WQ = [persist.tile([128, F], F16, tag=f"wq{c}", name=f"WQ{c}")
                  for c in range(CH)]
            WK = [persist.tile([128, F], F16, tag=f"wk{c}", name=f"WK{c}")
                  for c in range(CH)]
            WV = [persist.tile([128, F], F16, tag=f"wv{c}", name=f"WV{c}")
                  for c in range(CH)]
            WO = [persist.tile([128, OUTW], F16, tag=f"wo{c}", name=f"WO{c}")
                  for c in range(CH)]
            for c in (0, 1):
                nc.sync.dma_start(out=xT0[c],
                                  in_=xt[c * 128:(c + 1) * 128, 0:PANEL])
                nc.sync.dma_start(out=WQ[c], in_=wq[c * 128:(c + 1) * 128, :])
            bqt = consts.tile([128, FG], F32)
            nc.sync.dma_start(out=bqt, in_=bq.rearrange("(g p) -> p g", p=128))
            for c in range(2, CH):
                nc.sync.dma_start(out=xT0[c],
                                  in_=xt[c * 128:(c + 1) * 128, 0:PANEL])
                nc.sync.dma_start(out=WQ[c], in_=wq[c * 128:(c + 1) * 128, :])
            for c in range(CH):
                sl_c = slice(c * 128, (c + 1) * 128)
                nc.sync.dma_start(out=WK[c], in_=wk[sl_c, :])
                nc.sync.dma_start(out=WV[c], in_=wv[sl_c, :])

            # ---- constants ----
            tri = consts.tile([128, 128], F16)
            nc.sync.dma_start(out=tri, in_=tri_dram[:])
            bo_row = consts.tile([1, OUTW], F32)
            nc.sync.dma_start(out=bo_row,
                              in_=bo.rearrange("(a n) -> a n", a=1))
            bo_bc = consts.tile([128, OUTW], F32)
            nc.gpsimd.partition_broadcast(bo_bc, bo_row)

            for c in range(CH):
                nc.sync.dma_start(out=WO[c], in_=wo[c * 128:(c + 1) * 128, :])

            # ---- persistent attention operands ----
            KT = [persist.tile([128, sl], F16, tag=f"kt{f}", name=f"KT{f}")
                  for f in range(FG)]
            VP = [persist.tile([128, VW], F16, tag=f"vp{k}", name=f"VP{k}")
                  for k in range(NKB)]

            y_local = dram.tile([N_CC, F, SPAN], F16)
            # AllGather split into head-halves: half A (heads 0-3) fires
            # mid-panel so only half B's 256KB transfer sits on the tail
            y_allA = dram.tile([N_CC, 2, F // 2, SPAN], F16)
            y_allB = dram.tile([N_CC, 2, F // 2, SPAN], F16)

            QTs = {}
            y_rows = {}

            def emit_xT(p):
                tiles = [xtp.tile([128, PANEL], F16, tag=f"xt{c}",
                                  name=f"xT{c}") for c in range(CH)]
                for c in range(CH):
                    nc.sync.dma_start(
                        out=tiles[c],
                        in_=xt[c * 128:(c + 1) * 128,
                               p * PANEL:(p + 1) * PANEL])
                return tiles

            # ---------- projection micro-ops (2 matmuls each) ----------
            def proj_micros(p, xT):
                QT = [qtp.tile([128, PANEL], F16, tag=f"qt{f}",
                               name=f"QT{f}") for f in range(FG)]
                QTs[p] = QT
                micros = []

                def qk_mms(f, wtiles, st, c0):
                    def go():
                        if 'ps' not in st:
                            st['ps'] = psp.tile([128, PANEL], F32, tag="acc",
                                                bufs=2, name="ps_a")
                        for c in (c0, c0 + 1):
                            nc.tensor.matmul(
                                st['ps'], wtiles[c][:, f * 128:(f + 1) * 128],
                                xT[c], start=(c == 0), stop=(c == CH - 1))
                    return go

                def q_epi(f, st):
                    def go():
                        nc.vector.tensor_scalar_add(
                            QT[f], st['ps'], bqt[:, f:f + 1])
                    return go

                def k_epi(f, st):
                    def go():
                        nc.vector.tensor_copy(
                            KT[f][:, p * PANEL:(p + 1) * PANEL], st['ps'])
                    return go

                def v_mms(sub, st, c0):
                    def go():
                        if 'ps' not in st:
                            st['ps'] = psp.tile([128, F], F32, tag="acc",
                                                bufs=2, name="ps_v")
                        for c in (c0, c0 + 1):
                            nc.tensor.matmul(
                                st['ps'], xT[c][:, sub * 128:(sub + 1) * 128],
                                WV[c], start=(c == 0), stop=(c == CH - 1))
                    return go

                def v_epi(sub, st):
                    def go():
                        kb = p * 4 + sub
                        vp3 = VP[kb].rearrange("p (h e) -> p h e", e=65)
                        nc.vector.memset(vp3[:, :, 64:65], 1.0)
                        nc.vector.tensor_copy(
                            vp3[:, :, 0:64],
                            st['ps'].rearrange("p (h d) -> p h d", d=64))
                    return go

                for f in range(FG):
                    stq, stk = {}, {}
                    for c0 in range(0, CH, 2):
                        micros.append(qk_mms(f, WQ, stq, c0))
                    micros.append(q_epi(f, stq))
                    for c0 in range(0, CH, 2):
                        micros.append(qk_mms(f, WK, stk, c0))
                    micros.append(k_epi(f, stk))
                for sub in range(4):
                    stv = {}
                    for c0 in range(0, CH, 2):
                        micros.append(v_mms(sub, stv, c0))
                    micros.append(v_epi(sub, stv))
                return micros

            # ---------- attention slots (software-pipelined) ----------
            def head_slots(p, h):
                f, row = h // 2, (h % 2) * 64
                QT = QTs[p]
                nkb_p = 4 * p + 4
                npairs = nkb_p // 2
                st = {}

                def spair(j):
                    def go():
                        ps_s = psp.tile([128, 2 * PANEL], F32, tag="s",
                                        bufs=2, name="ps_s")
                        es = esp.tile([128, 2 * PANEL], F16, tag="es",
                                      name="es")
                        offs = []
                        for jj in (0, 1):
                            kb = 2 * j + jj
                            d = max(0, (kb - 4 * p) * 128)
                            offs.append((kb, d, PANEL - d, jj * PANEL))
                            nc.tensor.matmul(
                                ps_s[:, jj * PANEL:jj * PANEL + PANEL - d],
                                KT[f][row:row + 64,
                                      kb * 128:(kb + 1) * 128],
                                QT[f][row:row + 64, d:PANEL])
                        st[('es', j)] = es
                        st[('offs', j)] = offs
                        if 2 * j >= 4 * p:
                            for kb, d, n, o in offs:
                                nc.scalar.activation(
                                    es[:, o:o + n], ps_s[:, o:o + n],
                                    mybir.ActivationFunctionType.Exp)
                                nc.vector.tensor_mul(
                                    es[:, o:o + 128], es[:, o:o + 128], tri)
                        else:
                            nc.scalar.activation(
                                es, ps_s, mybir.ActivationFunctionType.Exp)
                    return go

                def pvpair(j):
                    def go():
                        if 'ps_y' not in st:
                            st['ps_y'] = psp.tile([65, PANEL], F32, tag="y",
                                                  bufs=2, name="ps_y")
                        ps_y = st['ps_y']
                        es = st.pop(('es', j))
                        for kb, d, n, o in st.pop(('offs', j)):
                            nc.tensor.matmul(
                                ps_y[:, d:PANEL],
                                VP[kb][:, h * 65:h * 65 + 65],
                                es[:, o:o + n],
                                start=(kb == 0), stop=(kb == nkb_p - 1))
                    return go

                def norm():
                    ps_y = st['ps_y']
                    den_s = misc.tile([1, PANEL], F32, tag="den",
                                      name="den")
                    nc.scalar.copy(den_s, ps_y[64:65, :])
                    recip = misc.tile([1, PANEL], F32, tag="recip",
                                      name="recip")
                    nc.vector.reciprocal_approx_fast(out=recip, in_=den_s)
                    bc = misc.tile([64, PANEL], F32, tag="bc", bufs=3,
                                   name="bc")
                    nc.gpsimd.partition_broadcast(bc, recip)
                    nY = nyp.tile([64, PANEL], F16, tag=f"ny{h}", name="nY")
                    nc.vector.tensor_mul(nY, ps_y[0:64, :], bc)
                    nc.sync.dma_start(
                        out=y_local[p, h * 64:(h + 1) * 64, :], in_=nY)

                slots = []
                for j in range(npairs):
                    slots.append(spair(j))
                    if j >= 2:
                        slots.append(pvpair(j - 2))
                slots.append(pvpair(npairs - 2))
                slots.append(pvpair(npairs - 1))
                slots.append(norm)
                return slots

            def attn_slots(p):
                slots = []
                for h in range(H):
                    slots += head_slots(p, h)
                return slots

            # ---------- collective + out-proj ----------
            def emit_cc_half(i, half):
                fh = F // 2
                y_in = y_local[i][half * fh:(half + 1) * fh, :]
                y_out = (y_allA if half == 0 else y_allB)[i]
                if fake_collective:
                    nc.sync.dma_start(out=y_out[0], in_=y_in)
                    nc.sync.dma_start(out=y_out[1], in_=y_in)
                else:
                    nc.gpsimd.collective_compute(
                        "AllGather",
                        mybir.AluOpType.bypass,
                        replica_groups=[[0, 1], [2, 3], [4, 5], [6, 7]],
                        ins=[y_in.opt()],
                        outs=[y_out.opt()],
                    )
                # one DMA for all 4 c-chunks of this half:
                # t[p, (g a), s] <- y_out[g, a*128+p, s]
                t = misc.tile([128, 4, PANEL], F16, tag=f"y_rows{half}",
                              bufs=2, name=f"y_rows{half}")
                nc.sync.dma_start(
                    out=t,
                    in_=y_out.rearrange("g (a p) s -> p (g a) s", p=128))
                # half 0 carries c = 0,1 (mine) and 4,5 (partner); half 1
                # carries c = 2,3 and 6,7
                cm = y_rows.setdefault(i, {})
                for j, c in enumerate((0, 1, 4, 5) if half == 0
                                      else (2, 3, 6, 7)):
                    cm[c] = (t, j)

            def outproj_micros(i):
                y_row = y_rows[i]
                micros = []

                def mms(sb4, st, c0):
                    def go():
                        if 'ps' not in st:
                            st['ps'] = psp.tile([128, OUTW], F32, tag="acc",
                                                bufs=2, name="ps_o")
                        for c in (c0, c0 + 1):
                            t, j = y_row[c]
                            nc.tensor.matmul(
                                st['ps'],
                                t[:, j, sb4 * 128:(sb4 + 1) * 128],
                                WO[c], start=(c == 0), stop=(c == CH - 1))
                    return go

                def epi(sb4, st):
                    def go():
                        sb = i * (PANEL // 128) + sb4
                        o_t = misc.tile([128, OUTW], F16, tag="o_t", bufs=3,
                                        name="o_t")
                        nc.vector.tensor_add(o_t, st['ps'], bo_bc)
                        # two DMAs land on separate queues -> 2x drain bw
                        r = slice(sb * 128, (sb + 1) * 128)
                        nc.sync.dma_start(out=out[r, 0:OUTW // 2],
                                          in_=o_t[:, 0:OUTW // 2])
                        nc.sync.dma_start(out=out[r, OUTW // 2:OUTW],
                                          in_=o_t[:, OUTW // 2:OUTW])
                    return go

                for sb4 in range(PANEL // 128):
                    st = {}
                    for c0 in range(0, CH, 2):
                        micros.append(mms(sb4, st, c0))
                    micros.append(epi(sb4, st))
                return micros

            def interleave(slots, fillers):
                n, m = len(slots), len(fillers)
                fi = 0
                for si, s in enumerate(slots):
                    s()
                    while fi < m and fi * n <= (si + 1) * m:
                        fillers[fi]()
                        fi += 1
                while fi < m:
                    fillers[fi]()
                    fi += 1

            # ---------- schedule ----------
            def run_round(pa, fillers):
                slotsA = [s for h in range(0, H // 2)
                          for s in head_slots(pa, h)]
                slotsB = [s for h in range(H // 2, H)
                          for s in head_slots(pa, h)]
                kA = len(fillers) * len(slotsA) // (len(slotsA) +
                                                    len(slotsB))
                interleave(slotsA, fillers[:kA])
                emit_cc_half(pa, 0)
                interleave(slotsB, fillers[kA:])
                emit_cc_half(pa, 1)
                QTs.pop(pa, None)

            # round 0: proj(0) straight
            for mo in proj_micros(0, xT0):
                mo()
            # rounds 1..NP-1: attn(p-1) x [proj(p) + outproj(p-2)]
            for p in range(1, NP):
                fillers = proj_micros(p, emit_xT(p))
                if p >= 2:
                    fillers += outproj_micros(p - 2)
                run_round(p - 1, fillers)
            # final round: attn(NP-1) x outproj(NP-2)
            run_round(NP - 1, outproj_micros(NP - 2))
            for mo in outproj_micros(NP - 1):
                mo()

    nc.compile()
    return nc


def shard_inputs(x, mask, Wqkv, bqkv, Wo, bo, sl=SL, ne=NE, nh=NH):
    """Host-side sharding: returns in_maps for the 8 cores."""
    H = nh // 2
    F = H * HD
    scale = 1.0 / np.sqrt(HD)
    bv_full = bqkv[2 * ne:3 * ne]
    bo_eff = bo + bv_full @ Wo  # V bias folded through the out projection
    in_maps = []
    for c in range(N_CORES):
        b, g = c // 2, c % 2
        qc = slice(g * F, (g + 1) * F)
        kc = slice(ne + g * F, ne + (g + 1) * F)
        vc = slice(2 * ne + g * F, 2 * ne + (g + 1) * F)
        oc = slice(g * (ne // 2), (g + 1) * (ne // 2))
        in_maps.append({
            "xt": np.ascontiguousarray(x[b].T).astype(np.float16),
            "wq": (np.ascontiguousarray(Wqkv[:, qc]) * scale).astype(np.float16),
            "wk": np.ascontiguousarray(Wqkv[:, kc]).astype(np.float16),
            "wv": np.ascontiguousarray(Wqkv[:, vc]).astype(np.float16),
            "bq": (np.ascontiguousarray(bqkv[qc]) * scale).astype(np.float32),
            "wo": np.ascontiguousarray(Wo[:, oc]).astype(np.float16),
            "bo": np.ascontiguousarray(bo_eff[oc]).astype(np.float32),
        })
    return in_maps


def unshard_output(results, sl=SL, ne=NE):
    out = np.empty((BS, sl, ne), dtype=np.float32)
    half = ne // 2
    for c in range(N_CORES):
        b, g = c // 2, c % 2
        out[b, :, g * half:(g + 1) * half] = results[c]["out"].astype(
            np.float32)
    return out


_NC_CACHE = {}


def kernel(x, mask, Wqkv, bqkv, Wo, bo):
    x = np.asarray(x, dtype=np.float32)
    Wqkv = np.asarray(Wqkv, dtype=np.float32)
    bqkv = np.asarray(bqkv, dtype=np.float32)
    Wo = np.asarray(Wo, dtype=np.float32)
    bo = np.asarray(bo, dtype=np.float32)
    if "nc" not in _NC_CACHE:
        _NC_CACHE["nc"] = build_nc()
    nc = _NC_CACHE["nc"]
    in_maps = shard_inputs(x, mask, Wqkv, bqkv, Wo, bo)
    res = run_bass_kernel_spmd(nc, in_maps, list(range(N_CORES)))
    return unshard_output(res.results)


# revision 17
# speedup vs baseline: 1.0843x; 1.0291x over previous
"""Causal self-attention (BS=4, SL=2048, NE=1024, NH=16) on 8 trn2 NeuronCores.

Sharding (uniform SPMD program on all 8 cores):
  core c -> batch b = c//2, head-group g = c%2 (8 of 16 heads, 512 feats).
  Each core: QKV proj for its heads (full 2048 rows of its batch),
  causal attention for its 8 heads, then pairwise AllGather of y
  (cores 2b/2b+1), and out-proj for a 512-column half of the output.
  Host reassembles [4, 2048, 1024] from the 8 [2048, 512] halves.

v2 optimizations over the first working version:
  - x is pre-transposed on the HOST, so every x^T tile is a plain row
    DMA (the DMA-transpose ring was 50%-busy serialized before).
  - K bias dropped (softmax is invariant to a per-query additive shift)
    and V bias folded into the out-proj bias on the host (y = sum w_k
    (v_k + bv) = sum w_k v_k + bv since sum w_k = 1).
  - softmax 1/denom via reciprocal_approx_fast (5x faster DVE op;
    the exact [1,512] reciprocal was 3.35us x32 = 107us of DVE busy).
  - out-proj bias via a precomputed broadcast tile + tensor_add epilogue
    instead of a [1,128]x[128,512] PE matmul per block.
  - software-pipelined attention inner loop (S-matmul pairs run two
    ahead of PV pairs, so PV never head-of-line-blocks the PE queue
    behind the Exp activation), with projection/out-proj micro-ops
    (2 matmuls each) interleaved between slots to fill PE gaps.
  - out-projection of chunk i is interleaved into attention of panel
    i+2 instead of running serialized at the end.
  - fp16 output (cast back to fp32 on host).

Matmul operands in fp16 (full PE rate, fp32 PSUM accumulate).
Attention computed in S^T = K @ Q^T layout so that:
  - PV needs no transposes: Y^T[65,q] += [V|1]^T @ expS^T (row 64 = denom)
  - softmax normalization via gpsimd partition_broadcast of 1/denom
"""

import sys

if "/opt/trn_rl_repo" not in sys.path:
    sys.path.insert(0, "/opt/trn_rl_repo")

import numpy as np

import concourse.bass as bass
import concourse.mybir as mybir
import concourse.tile as tile
from concourse import bacc
from concourse.bass_utils import run_bass_kernel_spmd

F32 = mybir.dt.float32
F16 = mybir.dt.float16

# problem dims (hardcoded per spec)
BS, SL, NE, NH = 4, 2048, 1024, 16
HD = 64
N_CORES = 8


def build_nc(sl=SL, ne=NE, nh=NH, fake_collective=False):
    """Build the per-core Bass program. All 8 cores run this identically."""
    H = nh // 2          # local heads per core
    F = H * HD           # local feats (q/k/v width per core)
    FG = F // 128        # feat groups of 128 (2 heads each)
    CH = ne // 128       # contraction chunks for the projections
    PANEL = 512          # q-panel width
    NP = sl // PANEL     # number of q panels
    NKB = sl // 128      # number of 128-row k blocks
    OUTW = ne // 2       # out-proj columns computed per core
    VW = H * 65          # V' width (65-stride per head: 64 V cols + ones)
    N_CC = NP            # collective chunk == q panel
    SPAN = PANEL

    nc = bacc.Bacc("TRN2", target_bir_lowering=False, num_devices=N_CORES)

    xt = nc.dram_tensor("xt", [ne, sl], F16, kind="ExternalInput")
    wq = nc.dram_tensor("wq", [ne, F], F16, kind="ExternalInput")
    wk = nc.dram_tensor("wk", [ne, F], F16, kind="ExternalInput")
    wv = nc.dram_tensor("wv", [ne, F], F16, kind="ExternalInput")
    bq = nc.dram_tensor("bq", [F], F32, kind="ExternalInput")
    wo = nc.dram_tensor("wo", [ne, OUTW], F16, kind="ExternalInput")
    bo = nc.dram_tensor("bo", [OUTW], F32, kind="ExternalInput")
    out = nc.dram_tensor("out", [sl, OUTW], F16, kind="ExternalOutput")

    tri_dram = nc.inline_tensor(
        np.triu(np.ones((128, 128), dtype=np.float16)), name="tri_c")

    with tile.TileContext(nc) as tc:
        with (
            tc.tile_pool(name="consts", bufs=1) as consts,
            tc.tile_pool(name="xt", bufs=3) as xtp,
            tc.tile_pool(name="qt", bufs=2) as qtp,
            tc.tile_pool(name="persist", bufs=1) as persist,
            tc.tile_pool(name="es", bufs=8) as esp,
            tc.tile_pool(name="ny", bufs=3) as nyp,
            tc.tile_pool(name="misc", bufs=2) as misc,
            tc.tile_pool(name="psum", bufs=1, space="PSUM") as psp,
            tc.tile_pool(name="dram", bufs=1, space="DRAM") as dram,
        ):
            # ---- panel-0 x^T tiles + first weights so proj starts ASAP ----
            xT0 = [xtp.tile([128, PANEL], F16, tag=f"xt{c}", name=f"xT0_{c}")
                   for c in range(CH)]
            WQ = [persist.tile([128, F], F16, tag=f"wq{c}", name=f"WQ{c}")
                  for c in range(CH)]
            WK = [persist.tile([128, F], F16, tag=f"wk{c}", name=f"WK{c}")
                  for c in range(CH)]
            WV = [persist.tile([128, F], F16, tag=f"wv{c}", name=f"WV{c}")
                  for c in range(CH)]
            WO = [persist.tile([128, OUTW], F16, tag=f"wo{c}", name=f"WO{c}")
                  for c in range(CH)]
            for c in (0, 1):
                nc.sync.dma_start(out=xT0[c],
                                  in_=xt[c * 128:(c + 1) * 128, 0:PANEL])
                nc.sync.dma_start(out=WQ[c], in_=wq[c * 128:(c + 1) * 128, :])
            bqt = consts.tile([128, FG], F32)
            nc.sync.dma_start(out=bqt, in_=bq.rearrange("(g p) -> p g", p=128))
            for c in range(2, CH):
                nc.sync.dma_start(out=xT0[c],
                                  in_=xt[c * 128:(c + 1) * 128, 0:PANEL])
                nc.sync.dma_start(out=WQ[c], in_=wq[c * 128:(c + 1) * 128, :])
            for c in range(CH):
                sl_c = slice(c * 128, (c + 1) * 128)
                nc.sync.dma_start(out=WK[c], in_=wk[sl_c, :])
                nc.sync.dma_start(out=WV[c], in_=wv[sl_c, :])

            # ---- constants ----
            tri = consts.tile([128, 128], F16)
            nc.sync.dma_start(out=tri, in_=tri_dram[:])
            bo_row = consts.tile([1, OUTW], F32)
            nc.sync.dma_start(out=bo_row,
                              in_=bo.rearrange("(a n) -> a n", a=1))
            bo_bc = consts.tile([128, OUTW], F32)
            nc.gpsimd.partition_broadcast(bo_bc, bo_row)

            for c in range(CH):
                nc.sync.dma_start(out=WO[c], in_=wo[c * 128:(c + 1) * 128, :])

            # ---- persistent attention operands ----
            KT = [persist.tile([128, sl], F16, tag=f"kt{f}", name=f"KT{f}")
                  for f in range(FG)]
            VP = [persist.tile([128, VW], F16, tag=f"vp{k}", name=f"VP{k}")
                  for k in range(NKB)]

            y_local = dram.tile([N_CC, F, SPAN], F16)
            # AllGather split into head-halves: half A (heads 0-3) fires
            # mid-panel so only half B's 256KB transfer sits on the tail
            y_allA = dram.tile([N_CC, 2, F // 2, SPAN], F16)
            y_allB = dram.tile([N_CC, 2, F // 2, SPAN], F16)

            QTs = {}
            y_rows = {}

            def emit_xT(p):
                tiles = [xtp.tile([128, PANEL], F16, tag=f"xt{c}",
                                  name=f"xT{c}") for c in range(CH)]
                for c in range(CH):
                    nc.sync.dma_start(
                        out=tiles[c],
                        in_=xt[c * 128:(c + 1) * 128,
                               p * PANEL:(p + 1) * PANEL])
                return tiles

            # ---------- projection micro-ops (2 matmuls each) ----------
            def proj_micros(p, xT):
                QT = [qtp.tile([128, PANEL], F16, tag=f"qt{f}",
                               name=f"QT{f}") for f in range(FG)]
                QTs[p] = QT
                micros = []

                def qk_mms(f, wtiles, st, c0):
                    def go():
                        if 'ps' not in st:
                            st['ps'] = psp.tile([128, PANEL], F32, tag="acc",
                                                bufs=2, name="ps_a")
                        for c in (c0, c0 + 1):
                            nc.tensor.matmul(
                                st['ps'], wtiles[c][:, f * 128:(f + 1) * 128],
                                xT[c], start=(c == 0), stop=(c == CH - 1))
                    return go

                def q_epi(f, st):
                    def go():
                        nc.vector.tensor_scalar_add(
                            QT[f], st['ps'], bqt[:, f:f + 1])
                    return go

                def k_epi(f, st):
                    def go():
                        nc.vector.tensor_copy(
                            KT[f][:, p * PANEL:(p + 1) * PANEL], st['ps'])
                    return go

                def v_mms(sub, st, c0):
                    def go():
                        if 'ps' not in st:
                            st['ps'] = psp.tile([128, F], F32, tag="acc",
                                                bufs=2, name="ps_v")
                        for c in (c0, c0 + 1):
                            nc.tensor.matmul(
                                st['ps'], xT[c][:, sub * 128:(sub + 1) * 128],
                                WV[c], start=(c == 0), stop=(c == CH - 1))
                    return go

                def v_epi(sub, st):
                    def go():
                        kb = p * 4 + sub
                        vp3 = VP[kb].rearrange("p (h e) -> p h e", e=65)
                        nc.vector.memset(vp3[:, :, 64:65], 1.0)
                        nc.vector.tensor_copy(
                            vp3[:, :, 0:64],
                            st['ps'].rearrange("p (h d) -> p h d", d=64))
                    return go

                for f in range(FG):
                    stq, stk = {}, {}
                    for c0 in range(0, CH, 2):
                        micros.append(qk_mms(f, WQ, stq, c0))
                    micros.append(q_epi(f, stq))
                    for c0 in range(0, CH, 2):
                        micros.append(qk_mms(f, WK, stk, c0))
                    micros.append(k_epi(f, stk))
                for sub in range(4):
                    stv = {}
                    for c0 in range(0, CH, 2):
                        micros.append(v_mms(sub, stv, c0))
                    micros.append(v_epi(sub, stv))
                return micros

            # ---------- attention slots (software-pipelined) ----------
            def head_slots(p, h):
                f, row = h // 2, (h % 2) * 64
                QT = QTs[p]
                nkb_p = 4 * p + 4
                npairs = nkb_p // 2
                st = {}

                def spair(j):
                    def go():
                        ps_s = psp.tile([128, 2 * PANEL], F32, tag="s",
                                        bufs=2, name="ps_s")
                        es = esp.tile([128, 2 * PANEL], F16, tag="es",
                                      name="es")
                        offs = []
                        for jj in (0, 1):
                            kb = 2 * j + jj
                            d = max(0, (kb - 4 * p) * 128)
                            offs.append((kb, d, PANEL - d, jj * PANEL))
                            nc.tensor.matmul(
                                ps_s[:, jj * PANEL:jj * PANEL + PANEL - d],
                                KT[f][row:row + 64,
                                      kb * 128:(kb + 1) * 128],
                                QT[f][row:row + 64, d:PANEL])
                        st[('es', j)] = es
                        st[('offs', j)] = offs
                        if 2 * j >= 4 * p:
                            for kb, d, n, o in offs:
                                nc.scalar.activation(
                                    es[:, o:o + n], ps_s[:, o:o + n],
                                    mybir.ActivationFunctionType.Exp)
                                nc.vector.tensor_mul(
                                    es[:, o:o + 128], es[:, o:o + 128], tri)
                        else:
                            nc.scalar.activation(
                                es, ps_s, mybir.ActivationFunctionType.Exp)
                    return go

                def pvpair(j):
                    def go():
                        if 'ps_y' not in st:
                            st['ps_y'] = psp.tile([65, PANEL], F32, tag="y",
                                                  bufs=2, name="ps_y")
                        ps_y = st['ps_y']
                        es = st.pop(('es', j))
                        for kb, d, n, o in st.pop(('offs', j)):
                            nc.tensor.matmul(
                                ps_y[:, d:PANEL],
                                VP[kb][:, h * 65:h * 65 + 65],
                                es[:, o:o + n],
                                start=(kb == 0), stop=(kb == nkb_p - 1))
                    return go

                def norm():
                    ps_y = st['ps_y']
                    den_s = misc.tile([1, PANEL], F32, tag="den",
                                      name="den")
                    # on DVE, not ACT: an ACT-side copy queues ahead of the
                    # next head's Exps and stalls its PV pairs ~1.3us
                    nc.vector.tensor_copy(den_s, ps_y[64:65, :])
                    recip = misc.tile([1, PANEL], F32, tag="recip",
                                      name="recip")
                    nc.vector.reciprocal_approx_fast(out=recip, in_=den_s)
                    bc = misc.tile([64, PANEL], F32, tag="bc", bufs=3,
                                   name="bc")
                    nc.gpsimd.partition_broadcast(bc, recip)
                    nY = nyp.tile([64, PANEL], F16, tag=f"ny{h}", name="nY")
                    nc.vector.tensor_mul(nY, ps_y[0:64, :], bc)
                    nc.sync.dma_start(
                        out=y_local[p, h * 64:(h + 1) * 64, :], in_=nY)

                slots = []
                for j in range(npairs):
                    slots.append(spair(j))
                    if j >= 2:
                        slots.append(pvpair(j - 2))
                slots.append(pvpair(npairs - 2))
                slots.append(pvpair(npairs - 1))
                slots.append(norm)
                return slots

            def attn_slots(p):
                slots = []
                for h in range(H):
                    slots += head_slots(p, h)
                return slots

            # ---------- collective + out-proj ----------
            def emit_cc_half(i, half):
                fh = F // 2
                y_in = y_local[i][half * fh:(half + 1) * fh, :]
                y_out = (y_allA if half == 0 else y_allB)[i]
                if fake_collective:
                    nc.sync.dma_start(out=y_out[0], in_=y_in)
                    nc.sync.dma_start(out=y_out[1], in_=y_in)
                else:
                    nc.gpsimd.collective_compute(
                        "AllGather",
                        mybir.AluOpType.bypass,
                        replica_groups=[[0, 1], [2, 3], [4, 5], [6, 7]],
                        ins=[y_in.opt()],
                        outs=[y_out.opt()],
                    )
                # one DMA for all 4 c-chunks of this half:
                # t[p, (g a), s] <- y_out[g, a*128+p, s]
                t = misc.tile([128, 4, PANEL], F16, tag=f"y_rows{half}",
                              bufs=2, name=f"y_rows{half}")
                nc.sync.dma_start(
                    out=t,
                    in_=y_out.rearrange("g (a p) s -> p (g a) s", p=128))
                # half 0 carries c = 0,1 (mine) and 4,5 (partner); half 1
                # carries c = 2,3 and 6,7
                cm = y_rows.setdefault(i, {})
                for j, c in enumerate((0, 1, 4, 5) if half == 0
                                      else (2, 3, 6, 7)):
                    cm[c] = (t, j)

            def outproj_micros(i):
                y_row = y_rows[i]
                micros = []

                def mms(sb4, st, c0):
                    def go():
                        if 'ps' not in st:
                            st['ps'] = psp.tile([128, OUTW], F32, tag="acc",
                                                bufs=2, name="ps_o")
                        for c in (c0, c0 + 1):
                            t, j = y_row[c]
                            nc.tensor.matmul(
                                st['ps'],
                                t[:, j, sb4 * 128:(sb4 + 1) * 128],
                                WO[c], start=(c == 0), stop=(c == CH - 1))
                    return go

                def epi(sb4, st):
                    def go():
                        sb = i * (PANEL // 128) + sb4
                        o_t = misc.tile([128, OUTW], F16, tag="o_t", bufs=3,
                                        name="o_t")
                        nc.vector.tensor_add(o_t, st['ps'], bo_bc)
                        # two DMAs land on separate queues -> 2x drain bw
                        r = slice(sb * 128, (sb + 1) * 128)
                        nc.sync.dma_start(out=out[r, 0:OUTW // 2],
                                          in_=o_t[:, 0:OUTW // 2])
                        nc.sync.dma_start(out=out[r, OUTW // 2:OUTW],
                                          in_=o_t[:, OUTW // 2:OUTW])
                    return go

                for sb4 in range(PANEL // 128):
                    st = {}
                    for c0 in range(0, CH, 2):
                        micros.append(mms(sb4, st, c0))
                    micros.append(epi(sb4, st))
                return micros

            def interleave(slots, fillers):
                n, m = len(slots), len(fillers)
                fi = 0
                for si, s in enumerate(slots):
                    s()
                    while fi < m and fi * n <= (si + 1) * m:
                        fillers[fi]()
                        fi += 1
                while fi < m:
                    fillers[fi]()
                    fi += 1

            # ---------- schedule ----------
            def run_round(pa, fillers):
                slotsA = [s for h in range(0, H // 2)
                          for s in head_slots(pa, h)]
                slotsB = [s for h in range(H // 2, H)
                          for s in head_slots(pa, h)]
                kA = len(fillers) * len(slotsA) // (len(slotsA) +
                                                    len(slotsB))
                interleave(slotsA, fillers[:kA])
                emit_cc_half(pa, 0)
                interleave(slotsB, fillers[kA:])
                emit_cc_half(pa, 1)
                QTs.pop(pa, None)

            # round 0: proj(0) straight
            for mo in proj_micros(0, xT0):
                mo()
            # rounds 1..NP-1: attn(p-1) x [proj(p) + outproj(p-2)]
            for p in range(1, NP):
                fillers = proj_micros(p, emit_xT(p))
                if p >= 2:
                    fillers += outproj_micros(p - 2)
                run_round(p - 1, fillers)
            # final round: attn(NP-1) x outproj(NP-2)
            run_round(NP - 1, outproj_micros(NP - 2))
            for mo in outproj_micros(NP - 1):
                mo()

    nc.compile()
    return nc


def shard_inputs(x, mask, Wqkv, bqkv, Wo, bo, sl=SL, ne=NE, nh=NH):
    """Host-side sharding: returns in_maps for the 8 cores."""
    H = nh // 2
    F = H * HD
    scale = 1.0 / np.sqrt(HD)
    bv_full = bqkv[2 * ne:3 * ne]
    bo_eff = bo + bv_full @ Wo  # V bias folded through the out projection
    in_maps = []
    for c in range(N_CORES):
        b, g = c // 2, c % 2
        qc = slice(g * F, (g + 1) * F)
        kc = slice(ne + g * F, ne + (g + 1) * F)
        vc = slice(2 * ne + g * F, 2 * ne + (g + 1) * F)
        oc = slice(g * (ne // 2), (g + 1) * (ne // 2))
        in_maps.append({
            "xt": np.ascontiguousarray(x[b].T).astype(np.float16),
            "wq": (np.ascontiguousarray(Wqkv[:, qc]) * scale).astype(np.float16),
            "wk": np.ascontiguousarray(Wqkv[:, kc]).astype(np.float16),
            "wv": np.ascontiguousarray(Wqkv[:, vc]).astype(np.float16),
            "bq": (np.ascontiguousarray(bqkv[qc]) * scale).astype(np.float32),
            "wo": np.ascontiguousarray(Wo[:, oc]).astype(np.float16),
            "bo": np.ascontiguousarray(bo_eff[oc]).astype(np.float32),
        })
    return in_maps


def unshard_output(results, sl=SL, ne=NE):
    out = np.empty((BS, sl, ne), dtype=np.float32)
    half = ne // 2
    for c in range(N_CORES):
        b, g = c // 2, c % 2
        out[b, :, g * half:(g + 1) * half] = results[c]["out"].astype(
            np.float32)
    return out


_NC_CACHE = {}


def kernel(x, mask, Wqkv, bqkv, Wo, bo):
    x = np.asarray(x, dtype=np.float32)
    Wqkv = np.asarray(Wqkv, dtype=np.float32)
    bqkv = np.asarray(bqkv, dtype=np.float32)
    Wo = np.asarray(Wo, dtype=np.float32)
    bo = np.asarray(bo, dtype=np.float32)
    if "nc" not in _NC_CACHE:
        _NC_CACHE["nc"] = build_nc()
    nc = _NC_CACHE["nc"]
    in_maps = shard_inputs(x, mask, Wqkv, bqkv, Wo, bo)
    res = run_bass_kernel_spmd(nc, in_maps, list(range(N_CORES)))
    return unshard_output(res.results)
